# revision 45
# baseline (speedup 1.0000x reference)
"""Trainium2 Bass kernel for nn_Block_21406117003497 (dense transformer block).

B=4, T=2048, C=1024, H=16 heads, HS=64, DFF=4096.
8 cores: core c -> batch c//2, token-half c%2 (causally balanced row split).

v4: fused layernorms with off-critical-path stats. The heavy projection
matmuls read RAW x (fp8 pairs for Q/K DoubleRow, bf16 for V); the LN
-mu*colsum(W) rank-1 term rides each psum group as its LAST (tiny) matmul,
so stats/row computation runs concurrently with the data matmuls and only
gates the psum->SBUF write (where rstd is applied: row-broadcast for Q/K,
per-partition rstd column for V via PE transposes of the rstd row). rstd is
computed as exp(-0.5*ln(var+eps)) so the whole kernel uses ONE activation
table set (square/ln/exp/copy) - zero table reloads. The causal mask is
additive (-1e3) folded into the scores psum via an identity-stationary
matmul on the PE, shortening the per-kt chain to scores->exp->AV.
MLP bf16. Residual path f32.
"""

import functools
from contextlib import ExitStack

import numpy as np
import ml_dtypes

import concourse.bass as bass
import concourse.mybir as mybir
import concourse.tile as tile
from concourse import bacc
from concourse.bass_utils import run_bass_kernel_spmd

F32 = mybir.dt.float32
BF16 = mybir.dt.bfloat16
FP8 = mybir.dt.float8e4
AF = mybir.ActivationFunctionType
ALU = mybir.AluOpType
DR = mybir.MatmulPerfMode.DoubleRow

B, T, C, H, HS = 4, 2048, 1024, 16, 64
DFF = 4 * C
R = 1024            # own rows per core
EPS = 1e-5
SCALE = float(C) ** -0.5
BF = ml_dtypes.bfloat16
E4 = ml_dtypes.float8_e4m3


def own_ranges(sub):
    """local row block -> absolute row ranges per sub (causally balanced)."""
    if sub == 0:
        return (0, 512), (1536, 2048)
    return (512, 1024), (1024, 1536)


def build_program(apply_ln_affine: bool, add_bproj: bool, add_b2: bool, repeat: int = 1,
                  loop_n: int = 0, variant: str = ""):
    nc = bacc.Bacc(None, target_bir_lowering=False, debug=False)

    env = {}
    env["variant"] = set(variant.split("+")) if variant else set()
    env["apply_ln_affine"] = apply_ln_affine
    env["add_bproj"] = add_bproj
    env["add_b2"] = add_b2
    # fp8 pair tiles for QK projections + stats: col = k2*(2T)+i*T+t
    env["x8_d"] = nc.dram_tensor("x8", [128, 4 * 2 * T], FP8, kind="ExternalInput")
    env["x8q_d"] = nc.dram_tensor("x8q", [128, 4 * 2 * R], FP8, kind="ExternalInput")
    # bf16 chunks for V stationary: col = k*T + t
    env["xb_d"] = nc.dram_tensor("xb", [128, 8 * T], BF16, kind="ExternalInput")
    env["xTq_d"] = nc.dram_tensor("xTq", [C, R], F32, kind="ExternalInput")
    # additive causal mask: 0 visible / -1000 hidden
    env["maskp_d"] = nc.dram_tensor("maskp", [128, 16 * 512], BF16, kind="ExternalInput")
    env["ident_d"] = nc.dram_tensor("ident", [128, 128], BF16, kind="ExternalInput")
    # DoubleRow weight packs: col = k2*2048 + i*1024 + (m*128+j)
    env["wq8_d"] = nc.dram_tensor("wq8", [128, 4 * 2 * C], FP8, kind="ExternalInput")
    env["wk8_d"] = nc.dram_tensor("wk8", [128, 4 * 2 * C], FP8, kind="ExternalInput")
    env["wv_d"] = nc.dram_tensor("wv", [128, 8 * C], BF16, kind="ExternalInput")
    env["wp_d"] = nc.dram_tensor("wp", [128, 8 * C], BF16, kind="ExternalInput")
    env["w1_d"] = nc.dram_tensor("w1", [128, 8 * DFF], BF16, kind="ExternalInput")
    env["w2_d"] = nc.dram_tensor("w2", [128, 32 * C], BF16, kind="ExternalInput")
    # colsum row (bf16 wv colsums, for the V write correction broadcast)
    env["cs_d"] = nc.dram_tensor("cs", [1, C], BF16, kind="ExternalInput")
    # beta rows (beta@W, b1 folded), used only when affine/bias nontrivial
    env["bw_d"] = nc.dram_tensor("bw", [1, 3 * C + DFF], BF16, kind="ExternalInput")
    # runtime fp8 weight scales [sq, sk] (refolded into the rstd rows)
    env["sqk_d"] = nc.dram_tensor("sqk", [1, 2], F32, kind="ExternalInput")
    # colsum columns: [cskc(8) | csqc(8) | cs1c(32)]
    env["csc_d"] = nc.dram_tensor("csc", [128, 48], BF16, kind="ExternalInput")
    env["bpT_d"] = nc.dram_tensor("bpT", [128, 2 * 8], F32, kind="ExternalInput")
    env["out_d"] = nc.dram_tensor("out", [C, R], F32, kind="ExternalOutput")

    with tile.TileContext(nc) as tc:
        with tc.tile_pool(name="consts", bufs=1, side="left") as consts:
            env["ones8"] = consts.tile([128, 2], FP8, name="ones8")
            nc.vector.memset(env["ones8"][:], 1.0)
            env["ones_b"] = consts.tile([128, 1], BF16, name="ones_b")
            nc.vector.memset(env["ones_b"][:], 1.0)
            env["one1"] = consts.tile([1, 1], F32, name="one1")
            nc.vector.memset(env["one1"][:], 1.0)
            env["eps_t"] = consts.tile([128, 1], F32, name="eps_t")
            nc.vector.memset(env["eps_t"][:], EPS)
            env["ident"] = consts.tile([128, 128], BF16, name="ident")
            nc.sync.dma_start(out=env["ident"][:], in_=env["ident_d"][:, :])
            env["cs_t"] = consts.tile([1, C], BF16, name="cs_t")
            nc.sync.dma_start(out=env["cs_t"][:], in_=env["cs_d"][:, :])
            env["sqk_t"] = consts.tile([1, 2], F32, name="sqk_t")
            nc.sync.dma_start(out=env["sqk_t"][:], in_=env["sqk_d"][:, :])
            env["csc_t"] = consts.tile([128, 48], BF16, name="csc_t")
            nc.sync.dma_start(out=env["csc_t"][:], in_=env["csc_d"][:, :])
            if apply_ln_affine:
                bw = consts.tile([1, 3 * C + DFF], BF16, name="bw_t")
                nc.sync.dma_start(out=bw[:], in_=env["bw_d"][:, :])
                env["bw_t"] = bw
            if add_bproj or add_b2:
                bpT = consts.tile([128, 16], F32, name="bpT_t")
                nc.sync.dma_start(out=bpT[:], in_=env["bpT_d"][:, :])
                env["bpT_t"] = bpT

            if loop_n:
                with tc.For_i(0, loop_n, 1):
                    emit_block(nc, tc, env)
            else:
                for _rep in range(repeat):
                    emit_block(nc, tc, env)
    nc.compile()
    return nc


def emit_block(nc, tc, env):
    V = env["variant"]
    affine = env["apply_ln_affine"]
    x8_d, x8q_d, xb_d, xTq_d = env["x8_d"], env["x8q_d"], env["xb_d"], env["xTq_d"]
    maskp_d = env["maskp_d"]
    wq8_d, wk8_d, wv_d, wp_d = env["wq8_d"], env["wk8_d"], env["wv_d"], env["wp_d"]
    w1_d, w2_d, out_d = env["w1_d"], env["w2_d"], env["out_d"]
    ones8, ones_b, one1, eps_t, cs_t, ident = (
        env["ones8"], env["ones_b"], env["one1"], env["eps_t"], env["cs_t"],
        env["ident"])
    csc_t = env["csc_t"]
    bw_t = env.get("bw_t")

    def cs_row(kind, sl):
        """colsum row slice: kind 0..2 = q/k/v chunks of C, 3 = cs1 (DFF)."""
        base = kind * C if kind < 3 else 3 * C
        return cs_t[0:1, base + sl.start: base + sl.stop]

    def bw_row(kind, sl):
        base = kind * C if kind < 3 else 3 * C
        return bw_t[0:1, base + sl.start: base + sl.stop]

    def ln_rows(ps_s, ps_q, p_row, rows, nb, bscale=None):
        """stats psums [1,512] -> rows. rstd = exp(-0.5*ln(var+eps)): stays in
        the ln/exp activation table set (no table reloads anywhere).

        bscale: optional [1,1] AP folded into the bf16 rstd row (fp8 weight
        scale compensation). Returns the f32 rstd row temp."""
        sl = slice(nb * 512, nb * 512 + 512)
        negmu_b, rstd_b, invstd_b, v_b = rows
        mu = p_row.tile([1, 512], F32, name="mu", tag="rowtmp")
        nc.vector.tensor_scalar(out=mu[:], in0=ps_s[:], scalar1=1.0 / C,
                                scalar2=None, op0=ALU.mult)
        nc.vector.tensor_scalar(out=negmu_b[0:1, sl], in0=ps_s[:], scalar1=-1.0 / C,
                                scalar2=None, op0=ALU.mult)
        nmu2 = p_row.tile([1, 512], F32, name="nmu2", tag="rowtmp")
        nc.vector.scalar_tensor_tensor(nmu2[:], mu[:], -1.0, mu[:],
                                       op0=ALU.mult, op1=ALU.mult)
        var = p_row.tile([1, 512], F32, name="var", tag="rowtmp")
        nc.vector.scalar_tensor_tensor(var[:], ps_q[:], 1.0 / C, nmu2[:],
                                       op0=ALU.mult, op1=ALU.add)
        lv = p_row.tile([1, 512], F32, name="lv", tag="rowtmp")
        nc.scalar.activation(lv[:], var[:], AF.Ln, bias=eps_t[0:1, 0:1])
        if invstd_b is not None:
            nc.scalar.activation(invstd_b[0:1, sl], lv[:], AF.Exp, scale=0.5)
        rtmp = p_row.tile([1, 512], F32, name="rstdtmp", tag="rowtmp")
        nc.scalar.activation(rtmp[:], lv[:], AF.Exp, scale=-0.5)
        if bscale is None:
            nc.vector.tensor_copy(rstd_b[0:1, sl], rtmp[:])
        else:
            nc.vector.tensor_scalar(out=rstd_b[0:1, sl], in0=rtmp[:],
                                    scalar1=bscale, scalar2=None, op0=ALU.mult)
        if v_b is not None:
            nc.vector.tensor_mul(v_b[0:1, sl], negmu_b[0:1, sl], rstd_b[0:1, sl])
        return rtmp

    def ln_stats_fp8(x8t, Wtot, ps_pool, p_sq, rows, p_row, sfx, bscale=None,
                     col_out=None, ps_rcp=None, vcol_out=None, negmu_for_v=None):
        """ones-matmul stats over fp8 pair tiles -> rows via ln_rows.

        col_out: optional [128, Wtot//128] tile receiving rstd in column form
        (PE transposes of the f32 rstd row, for per-partition V scaling)."""
        nbs = Wtot // 512
        o1 = ones8[:, 0:1]
        for nb in range(nbs):
            sl = slice(nb * 512, nb * 512 + 512)
            st_s = ps_pool.tile([1, 512], F32, name=f"st{sfx}_s", tag="st")
            st_q = ps_pool.tile([1, 512], F32, name=f"st{sfx}_q", tag="st")
            for k2 in range(4):
                xv = x8t[k2].rearrange("p (i t) -> p i t", i=2)[:, :, sl]
                sq = p_sq.tile([128, 2 * 512], FP8, name=f"sq{sfx}", tag=f"sq{sfx}")
                sqv = sq.rearrange("p (i t) -> p i t", i=2)
                nc.scalar.activation(sqv, xv, AF.Square)
                for i in range(2):
                    st = (k2 == 0 and i == 0)
                    sp = (k2 == 3 and i == 1)
                    nc.tensor.matmul(st_s[:], o1, xv[:, i, :], start=st, stop=sp)
                    nc.tensor.matmul(st_q[:], o1, sqv[:, i, :], start=st, stop=sp)
            rtmp = ln_rows(st_s, st_q, p_row, rows, nb, bscale=bscale)
            if col_out is not None:
                vtmp = p_row.tile([1, 512], F32, name="vtmp", tag="rowtmp")
                nc.vector.tensor_mul(vtmp[:], negmu_for_v[0:1, sl], rtmp[:])
                for i in range(4):
                    ps_rc = ps_rcp.tile([128, 1], F32, name="ps_rc", tag="strc")
                    nc.tensor.transpose(ps_rc[:], rtmp[0:1, i * 128:(i + 1) * 128],
                                        one1[:])
                    nc.vector.tensor_copy(col_out[:, nb * 4 + i:nb * 4 + i + 1],
                                          ps_rc[:])
                    ps_rc2 = ps_rcp.tile([128, 1], F32, name="ps_rc2", tag="strc")
                    nc.tensor.transpose(ps_rc2[:], vtmp[0:1, i * 128:(i + 1) * 128],
                                        one1[:])
                    nc.vector.tensor_copy(vcol_out[:, nb * 4 + i:nb * 4 + i + 1],
                                          ps_rc2[:])

    # ================= Phase A: DMA x, LN1 stats ============================
    es_rows = ExitStack()
    p_rows = es_rows.enter_context(tc.tile_pool(name="p_rows", bufs=1, side="left"))
    p_bc = es_rows.enter_context(tc.tile_pool(name="p_bc", bufs=1, side="left"))

    es_x = ExitStack()
    p_rows1 = es_x.enter_context(tc.tile_pool(name="p_rows1", bufs=1, side="left"))
    p_bc1 = es_x.enter_context(tc.tile_pool(name="p_bc1", bufs=1, side="left"))
    p_x8 = es_x.enter_context(tc.tile_pool(name="p_x8", bufs=4, side="left"))
    p_x8q = es_x.enter_context(tc.tile_pool(name="p_x8q", bufs=4, side="left"))
    p_xb = es_x.enter_context(tc.tile_pool(name="p_xb", bufs=8, side="left"))

    x8 = [p_x8.tile([128, 2 * T], FP8, name=f"x8_{k2}", tag="x8") for k2 in range(4)]
    x8q = [p_x8q.tile([128, 2 * R], FP8, name=f"x8q_{k2}", tag="x8q") for k2 in range(4)]
    xb = [p_xb.tile([128, T], BF16, name=f"xb{k}", tag="xb") for k in range(8)]
    for k2 in range(4):
        nc.sync.dma_start(out=x8[k2][:], in_=x8_d[:, k2 * 2 * T:(k2 + 1) * 2 * T])
        nc.sync.dma_start(out=x8q[k2][:], in_=x8q_d[:, k2 * 2 * R:(k2 + 1) * 2 * R])
    for k in range(8):
        nc.sync.dma_start(out=xb[k][:], in_=xb_d[:, k * T:(k + 1) * T])

    # LN1 rows (phase A/B only)
    negmu1 = p_rows1.tile([1, T], BF16, name="negmu1")
    rstd1b = p_rows1.tile([1, T], BF16, name="rstd1b")
    invstd1 = p_rows1.tile([1, T], BF16, name="invstd1") if affine else None
    vk_r = p_rows1.tile([1, T], BF16, name="vk_r")
    negmu1q = p_rows1.tile([1, R], BF16, name="negmu1q")
    rstd1qb = p_rows1.tile([1, R], BF16, name="rstd1qb")
    invstd1q = p_rows1.tile([1, R], BF16, name="invstd1q") if affine else None
    vq_r = p_rows1.tile([1, R], BF16, name="vq_r")
    rstd_col = p_rows1.tile([128, 16], F32, name="rstd_col")
    v_col = p_rows1.tile([128, 16], F32, name="v_col")
    rstdb1 = p_bc1.tile([128, T], BF16, name="rstdb1")
    rstdb1q = p_bc1.tile([128, R], BF16, name="rstdb1q")
    vkb = p_bc1.tile([128, T], BF16, name="vkb")
    vqb = p_bc1.tile([128, R], BF16, name="vqb")
    csvb = p_bc1.tile([128, 2 * 512], BF16, name="csvb")
    # LN2 rows (live into phase D)
    negmu2 = p_rows.tile([1, R], BF16, name="negmu2")
    rstd2b = p_rows.tile([1, R], BF16, name="rstd2b")
    invstd2 = p_rows.tile([1, R], BF16, name="invstd2") if affine else None
    rstdb2 = p_bc.tile([128, R], BF16, name="rstdb2")
    negmub2 = p_bc.tile([128, R], BF16, name="negmub2")

    sqk_t = env["sqk_t"]
    if "nostats" in V:
        for t_ in (rstdb1, rstdb1q, rstd_col):
            nc.vector.memset(t_[:], 1.0)
        for t_ in (vkb, vqb, v_col, csvb):
            nc.vector.memset(t_[:], 0.0)
    else:
        with tc.tile_pool(name="ps_st", bufs=4, space="PSUM") as ps_st, \
             tc.tile_pool(name="ps_rcp", bufs=2, space="PSUM") as ps_rcp, \
             tc.tile_pool(name="p_sq8", bufs=3, side="right") as p_sq8, \
             tc.tile_pool(name="p_lnrow", bufs=4, side="right") as p_row:
            ln_stats_fp8(x8, T, ps_st, p_sq8, (negmu1, rstd1b, invstd1, vk_r),
                         p_row, "", bscale=sqk_t[0:1, 1:2],
                         col_out=rstd_col, ps_rcp=ps_rcp, vcol_out=v_col,
                         negmu_for_v=negmu1)
            ln_stats_fp8(x8q, R, ps_st, p_sq8, (negmu1q, rstd1qb, invstd1q, vq_r),
                         p_row, "q", bscale=sqk_t[0:1, 0:1])
            nc.gpsimd.partition_broadcast(rstdb1[:], rstd1b[0:1, :])
            nc.gpsimd.partition_broadcast(rstdb1q[:], rstd1qb[0:1, :])
            nc.gpsimd.partition_broadcast(vkb[:], vk_r[0:1, :])
            nc.gpsimd.partition_broadcast(vqb[:], vq_r[0:1, :])
            nc.gpsimd.partition_broadcast(csvb[:], cs_t[0:1, :])

    # ================= Phase B: QKV projections =============================
    es_qkv = ExitStack()
    p_QT = es_qkv.enter_context(tc.tile_pool(name="p_QT", bufs=16, side="right"))
    p_KT = es_qkv.enter_context(tc.tile_pool(name="p_KT", bufs=8, side="right"))
    p_V = es_qkv.enter_context(tc.tile_pool(name="p_V", bufs=16, side="right"))
    QTb = [[p_QT.tile([128, 512], BF16, name=f"QT{b}_{m}", tag="QT")
            for m in range(8)] for b in range(2)]
    KT = [p_KT.tile([128, T], BF16, name=f"KT{m}", tag="KT") for m in range(8)]
    Vg = [p_V.tile([128, 2 * 16 * 65], FP8, name=f"Vg{i}", tag="Vg") for i in range(8)]

    with tc.tile_pool(name="p_w8", bufs=4, side="left") as p_w8, \
         tc.tile_pool(name="p_wv", bufs=1, side="left") as p_wv, \
         tc.tile_pool(name="ps_qkv", bufs=6, space="PSUM") as ps_a:
        if "noqkv" in V:
            for t_ in QTb[0] + QTb[1] + KT + Vg:
                nc.vector.memset(t_[:, :], 0.01)
        else:
            # K projection (full T): fp8 DoubleRow; -mu*cs rank-1 rides LAST
            wk8 = [p_w8.tile([128, 2 * C], FP8, name=f"wk8_{k2}", tag="w8")
                   for k2 in range(4)]
            for k2 in range(4):
                nc.sync.dma_start(out=wk8[k2][:],
                                  in_=wk8_d[:, k2 * 2 * C:(k2 + 1) * 2 * C])
            wk8v = [w.rearrange("p (i m) -> p i m", i=2) for w in wk8]
            x8v = [w.rearrange("p (i t) -> p i t", i=2) for w in x8]
            x8qv = [w.rearrange("p (i t) -> p i t", i=2) for w in x8q]
            for m in range(8):
                msl = slice(m * 128, (m + 1) * 128)
                for nb in range(4):
                    sl = slice(nb * 512, (nb + 1) * 512)
                    ps = ps_a.tile([128, 512], F32, name="k_ps", tag="ps_a")
                    for k2 in range(4):
                        nc.tensor.matmul(ps[:], wk8v[k2][:, :, msl],
                                         x8v[k2][:, :, sl],
                                         start=(k2 == 0),
                                         stop=(k2 == 3 and not affine), perf_mode=DR)
                    if affine:
                        nc.tensor.matmul(ps[:], bw_row(1, msl), invstd1[0:1, sl],
                                         start=False, stop=True)
                    nc.vector.tensor_mul(KT[m][:, sl], ps[:], rstdb1[:, sl])
                    nc.vector.scalar_tensor_tensor(
                        KT[m][:, sl], vkb[:, sl], csc_t[:, m:m + 1],
                        KT[m][:, sl], op0=ALU.mult, op1=ALU.add)
            # Q projection (own rows)
            wq8 = [p_w8.tile([128, 2 * C], FP8, name=f"wq8_{k2}", tag="w8")
                   for k2 in range(4)]
            for k2 in range(4):
                nc.sync.dma_start(out=wq8[k2][:],
                                  in_=wq8_d[:, k2 * 2 * C:(k2 + 1) * 2 * C])
            wq8v = [w.rearrange("p (i m) -> p i m", i=2) for w in wq8]
            for m in range(8):
                msl = slice(m * 128, (m + 1) * 128)
                for b in range(2):
                    sl = slice(b * 512, (b + 1) * 512)
                    ps = ps_a.tile([128, 512], F32, name="q_ps", tag="ps_a")
                    for k2 in range(4):
                        nc.tensor.matmul(ps[:], wq8v[k2][:, :, msl],
                                         x8qv[k2][:, :, sl],
                                         start=(k2 == 0),
                                         stop=(k2 == 3 and not affine), perf_mode=DR)
                    if affine:
                        nc.tensor.matmul(ps[:], bw_row(0, msl), invstd1q[0:1, sl],
                                         start=False, stop=True)
                    nc.vector.tensor_mul(QTb[b][m][:], ps[:], rstdb1q[:, sl])
                    nc.vector.scalar_tensor_tensor(
                        QTb[b][m][:], vqb[:, sl], csc_t[:, 8 + m:8 + m + 1],
                        QTb[b][m][:], op0=ALU.mult, op1=ALU.add)
            # V projection (full T, bf16): stationary xb chunk, moving wv
            wv = p_wv.tile([128, 8 * C], BF16, name="wv_t", tag="wv")
            nc.sync.dma_start(out=wv[:], in_=wv_d[:, :])
            for tch in range(16):
                tsl = slice(tch * 128, (tch + 1) * 128)
                if tch % 2 == 0:
                    nc.gpsimd.memset(
                        Vg[tch // 2][:].rearrange("p (i h d) -> p i h d", i=2, d=65)[:, :, :, 64:65], 1.0)
                psA = ps_a.tile([128, 512], F32, name="v_psA", tag="ps_a")
                psB = ps_a.tile([128, 512], F32, name="v_psB", tag="ps_a")
                for k in range(8):
                    lhs = xb[k][:, tsl]
                    sp = (k == 7 and not affine)
                    nc.tensor.matmul(psA[:], lhs, wv[:, k * C:k * C + 512],
                                     start=(k == 0), stop=sp)
                    nc.tensor.matmul(psB[:], lhs, wv[:, k * C + 512:k * C + 1024],
                                     start=(k == 0), stop=sp)
                if affine:
                    nc.tensor.matmul(psA[:], invstd1[0:1, tsl],
                                     bw_row(2, slice(0, 512)), start=False, stop=True)
                    nc.tensor.matmul(psB[:], invstd1[0:1, tsl],
                                     bw_row(2, slice(512, 1024)), start=False, stop=True)
                for j, ps in ((0, psA), (1, psB)):
                    vi = tch % 2
                    base = vi * 16 * 65 + j * 8 * 65
                    dst = Vg[tch // 2][:, base:base + 8 * 65].rearrange(
                        "p (h d) -> p h d", d=65)[:, :, 0:64]
                    nc.vector.tensor_scalar(
                        out=dst, in0=ps[:].rearrange("p (h d) -> p h d", d=64),
                        scalar1=rstd_col[:, tch:tch + 1], scalar2=None, op0=ALU.mult)
                    csvv = csvb[:, j * 512:(j + 1) * 512].rearrange(
                        "p (h d) -> p h d", d=64)
                    nc.vector.scalar_tensor_tensor(
                        dst, csvv, v_col[:, tch:tch + 1], dst,
                        op0=ALU.mult, op1=ALU.add)
    es_x.close()  # x8/x8q/xb + LN1 rows freed

    # ================= Phase C: attention (b outer) + proj + LN2 ============
    es_x2 = ExitStack()
    p_x2 = es_x2.enter_context(tc.tile_pool(name="p_x2", bufs=8, side="left"))
    p_x2b = es_x2.enter_context(tc.tile_pool(name="p_x2b", bufs=8, side="left"))
    x2T = [p_x2.tile([128, R], F32, name=f"x2_{c}", tag="x2") for c in range(8)]
    x2b = [p_x2b.tile([128, R], BF16, name=f"x2b{c}", tag="x2b") for c in range(8)]

    LAG = 4
    es_att = ExitStack()
    p_oT = es_att.enter_context(tc.tile_pool(name="p_oT", bufs=16, side="left"))
    p_mask = es_att.enter_context(tc.tile_pool(name="p_mask", bufs=8, side="right"))
    p_E = es_att.enter_context(tc.tile_pool(name="p_E", bufs=6, side="right"))
    p_inv = es_att.enter_context(tc.tile_pool(name="p_inv", bufs=2, side="right"))
    es_attps = ExitStack()
    ps_s = es_attps.enter_context(tc.tile_pool(name="ps_s", bufs=3, space="PSUM"))
    ps_av = es_attps.enter_context(tc.tile_pool(name="ps_av", bufs=2, space="PSUM"))

    oTb = [[p_oT.tile([128, 512], BF16, name=f"oT{b}_{m}", tag="oT")
            for m in range(8)] for b in range(2)]

    def attn_b(b):
        n_kt = 8 if b == 0 else 16
        mk = []
        for mi in range(8):
            mt = p_mask.tile([128, 512], BF16, name=f"mk{mi}", tag="mk")
            nc.sync.dma_start(out=mt[:],
                              in_=maskp_d[:, (b * 8 + mi) * 512:(b * 8 + mi + 1) * 512])
            mk.append(mt)
        for hp in range(8):
            qslc = [QTb[b][hp][hh * 64:(hh + 1) * 64, :] for hh in (0, 1)]
            oa = ps_av.tile([128, 512], F32, name="av_psA", tag="ps_av")
            ob = ps_av.tile([128, 512], F32, name="av_psB", tag="ps_av")
            Es = []
            for kt in range(n_kt + LAG):
                if kt < n_kt:
                    masked = ((b == 0) or (kt >= 8)) and "nomask" not in V
                    sps = ps_s.tile([128, 1024], F32, name="s_ps", tag="ps_s")
                    mi = kt if b == 0 else kt - 8
                    for hh in (0, 1):
                        kslc = KT[hp][hh * 64:(hh + 1) * 64, kt * 128:(kt + 1) * 128]
                        nc.tensor.matmul(sps[:, hh * 512:(hh + 1) * 512],
                                         kslc, qslc[hh], start=True, stop=not masked)
                        if masked:
                            # additive -1e3 causal mask via identity-stationary
                            nc.tensor.matmul(sps[:, hh * 512:(hh + 1) * 512],
                                             ident[:], mk[mi][:],
                                             start=False, stop=True)
                    if kt % 2 == 0:
                        Es.append(p_E.tile([128, 2 * 1024], FP8, name="E", tag="E"))
                    Ev = Es[kt // 2].rearrange("p (i t) -> p i t", i=2)
                    nc.scalar.activation(Ev[:, kt % 2, :], sps[:], AF.Exp, scale=SCALE)
                kta = kt - LAG
                if 0 <= kta and kta % 2 == 1:
                    h0 = 2 * hp
                    k2a = kta // 2
                    Vv = Vg[k2a].rearrange("p (i hd) -> p i hd", i=2)
                    Ev = Es[k2a].rearrange("p (i hd) -> p i hd", i=2)
                    nc.tensor.matmul(oa[0:65, :],
                                     Vv[:, :, h0 * 65:h0 * 65 + 65],
                                     Ev[:, :, 0:512],
                                     start=(kta == 1), stop=(kta == n_kt - 1),
                                     perf_mode=DR)
                    nc.tensor.matmul(ob[0:65, :],
                                     Vv[:, :, (h0 + 1) * 65:(h0 + 1) * 65 + 65],
                                     Ev[:, :, 512:1024],
                                     start=(kta == 1), stop=(kta == n_kt - 1),
                                     perf_mode=DR)
            for hh, ops in ((0, oa), (1, ob)):
                invd = p_inv.tile([1, 512], F32, name="invd", tag="invd")
                nc.vector.reciprocal(invd[:], ops[64:65, :])
                invb = p_inv.tile([64, 512], F32, name="invb", tag="invb")
                nc.gpsimd.partition_broadcast(invb[:], invd[0:1, :])
                nc.vector.tensor_mul(oTb[b][hp][hh * 64:(hh + 1) * 64, :],
                                     ops[0:64, :], invb[:])

    def proj_ln2_b(b):
        bsl = slice(b * 512, (b + 1) * 512)
        # proj^T + residual -> x2T[:, b-half] (f32) and x2b (bf16)
        with tc.tile_pool(name="ps_pj", bufs=2, space="PSUM") as ps_pj, \
             tc.tile_pool(name="p_xo", bufs=3, side="right") as p_xo, \
             tc.tile_pool(name="p_wp", bufs=3, side="right") as p_wp:
            for m in range(8):
                wpt = p_wp.tile([128, C], BF16, name="wp_t", tag="wp")
                nc.sync.dma_start(out=wpt[:], in_=wp_d[:, m * C:(m + 1) * C])
                xo = p_xo.tile([128, 512], F32, name="xo", tag="xo")
                nc.sync.dma_start(out=xo[:], in_=xTq_d[m * 128:(m + 1) * 128, bsl])
                ps = ps_pj.tile([128, 512], F32, name="pj_ps", tag="ps_pj")
                for k in range(8):
                    lhs = wpt[:, k * 128:(k + 1) * 128]
                    nc.tensor.matmul(ps[:], lhs, oTb[b][k][:, :],
                                     start=(k == 0), stop=(k == 7))
                nc.vector.scalar_tensor_tensor(x2T[m][:, bsl], ps[:], 1.0, xo[:],
                                               op0=ALU.mult, op1=ALU.add)
                if env["add_bproj"]:
                    nc.vector.tensor_scalar(out=x2T[m][:, bsl], in0=x2T[m][:, bsl],
                                            scalar1=env["bpT_t"][:, m:m + 1],
                                            scalar2=None, op0=ALU.add)
                eng = nc.vector if m % 2 == 0 else nc.gpsimd
                eng.tensor_copy(x2b[m][:, bsl], x2T[m][:, bsl])
        # LN2 stats on x2b[:, b-half] (bf16 ones-matmuls)
        with tc.tile_pool(name="ps_pj2", bufs=2, space="PSUM") as ps_pj2, \
             tc.tile_pool(name="p_l2sq", bufs=3, side="right") as p_sq2, \
             tc.tile_pool(name="p_l2row", bufs=4, side="right") as p_row2:
            st_s = ps_pj2.tile([1, 512], F32, name="st2_s", tag="ps_pj")
            st_q = ps_pj2.tile([1, 512], F32, name="st2_q", tag="ps_pj")
            for c in range(8):
                nc.tensor.matmul(st_s[:], ones_b[:], x2b[c][:, bsl],
                                 start=(c == 0), stop=(c == 7))
                sq = p_sq2.tile([128, 512], BF16, name="sq2", tag="sq2")
                nc.scalar.activation(sq[:], x2b[c][:, bsl], AF.Square)
                nc.tensor.matmul(st_q[:], ones_b[:], sq[:],
                                 start=(c == 0), stop=(c == 7))
            ln_rows(st_s, st_q, p_row2, (negmu2, rstd2b, invstd2, None), b)
            nc.gpsimd.partition_broadcast(rstdb2[:, bsl], rstd2b[0:1, bsl])
            nc.gpsimd.partition_broadcast(negmub2[:, bsl], negmu2[0:1, bsl])

    if "noattn" in V:
        for bl in oTb:
            for t_ in bl:
                nc.vector.memset(t_[:, :], 0.01)
        es_attps.close()
    else:
        attn_b(0)
        attn_b(1)
        es_attps.close()
    if "noproj" in V:
        for t_ in x2T:
            nc.vector.memset(t_[:, :], 0.01)
        for t_ in x2b:
            nc.vector.memset(t_[:, :], 0.5)
        nc.vector.memset(rstdb2[:], 1.0)
        nc.vector.memset(negmub2[:], 0.0)
        nc.vector.memset(negmu2[:], 0.0)
    else:
        proj_ln2_b(0)
        proj_ln2_b(1)
    es_att.close()
    es_qkv.close()  # QT/KT/Vg freed before MLP needs the space

    # ================= Phase D: MLP =========================================
    es_r1 = ExitStack()
    p_r1 = es_r1.enter_context(tc.tile_pool(name="p_r1", bufs=32, side="right"))
    r1T = [p_r1.tile([128, R], BF16, name=f"r1T{g}", tag="r1T") for g in range(32)]
    ps_m = es_r1.enter_context(tc.tile_pool(name="ps_m", bufs=6, space="PSUM"))

    with tc.tile_pool(name="p_w1", bufs=3, side="left") as p_w1, \
         tc.tile_pool(name="p_w2", bufs=2, side="left") as p_w2, \
         tc.tile_pool(name="p_rt", bufs=4, side="left") as p_rt:
        for db in (() if "nomlp" in V else range(8)):
            w1t = p_w1.tile([128, DFF], BF16, name=f"w1t{db}", tag="w1t")
            nc.sync.dma_start(out=w1t[:], in_=w1_d[:, db * DFF:(db + 1) * DFF])
            for dc in range(4):
                g = db * 4 + dc
                gsl = slice(g * 128, (g + 1) * 128)
                for j in range(2):
                    jsl = slice(j * 512, (j + 1) * 512)
                    ps = ps_m.tile([128, 512], F32, name="m1_ps", tag="ps_m")
                    for k in range(8):
                        lhs = w1t[:, k * 512 + dc * 128:k * 512 + (dc + 1) * 128]
                        nc.tensor.matmul(ps[:], lhs, x2b[k][:, jsl],
                                         start=(k == 0),
                                         stop=(k == 7 and not affine))
                    if affine:
                        nc.tensor.matmul(ps[:], bw_row(3, gsl), invstd2[0:1, jsl],
                                         start=False, stop=True)
                    # raw' = raw + cs1_g * (-mu2_t); r1 = rstd2 * relu(raw')
                    rt = p_rt.tile([128, 512], BF16, name="m1_rt", tag="m1rt")
                    nc.vector.scalar_tensor_tensor(
                        rt[:], negmub2[:, jsl], csc_t[:, 16 + g:16 + g + 1],
                        ps[:], op0=ALU.mult, op1=ALU.add)
                    nc.vector.scalar_tensor_tensor(
                        r1T[g][:, jsl], rt[:], 0.0, rstdb2[:, jsl],
                        op0=ALU.max, op1=ALU.mult)

        for q in range(4) if "nomlp" not in V else ():
            w2t = p_w2.tile([128, 8 * C], BF16, name=f"w2t{q}", tag="w2t")
            nc.sync.dma_start(out=w2t[:], in_=w2_d[:, q * 8 * C:(q + 1) * 8 * C])
            for m in range(8):
                for rb in range(2):
                    sl = slice(rb * 512, (rb + 1) * 512)
                    ps = ps_m.tile([128, 512], F32, name="m2_ps", tag="ps_m")
                    for kl in range(8):
                        lhs = w2t[:, kl * C + m * 128:kl * C + (m + 1) * 128]
                        nc.tensor.matmul(ps[:], lhs, r1T[q * 8 + kl][:, sl],
                                         start=(kl == 0), stop=(kl == 7))
                    nc.vector.scalar_tensor_tensor(x2T[m][:, sl], ps[:], 1.0,
                                                   x2T[m][:, sl], op0=ALU.mult, op1=ALU.add)
                if q == 3:
                    if env["add_b2"]:
                        nc.vector.tensor_scalar(out=x2T[m][:, :], in0=x2T[m][:, :],
                                                scalar1=env["bpT_t"][:, 8 + m:8 + m + 1],
                                                scalar2=None, op0=ALU.add)
                    nc.sync.dma_start(out=out_d[m * 128:(m + 1) * 128, :], in_=x2T[m][:])

    es_r1.close()
    es_x2.close()
    es_rows.close()


@functools.lru_cache(maxsize=16)
def _cached_program(apply_ln_affine, add_bproj, add_b2, repeat, loop_n=0, variant=""):
    return build_program(apply_ln_affine, add_bproj, add_b2, repeat, loop_n, variant)


def _pack_rows(w):
    """[8k*128, N] -> [128, 8k*N] with col = k*N + j."""
    kchunks = w.shape[0] // 128
    return np.ascontiguousarray(
        w.reshape(kchunks, 128, w.shape[1]).transpose(1, 0, 2).reshape(128, -1)
    ).astype(BF)


def _pack_dr(w):
    """[C, M] -> scaled fp8 DoubleRow pack [128, 4*2*M]: col = k2*2M + i*M + m.

    Returns (pack, scale): pack holds w/scale (e4m3 IEEE max is 240; raw
    transformer weights ~0.02 would land in subnormals without the scale).
    """
    M = w.shape[1]
    s = float(np.abs(w).max()) / 192.0
    s = max(s, 1e-20)
    pack = np.ascontiguousarray(
        (w / s).reshape(4, 2, 128, M).transpose(2, 0, 1, 3).reshape(128, 8 * M)
    ).astype(E4)
    return pack, s


def _pack_pairs_fp8(xT):
    """[C, N] -> fp8 pair tiles [128, 4*2*N]: col = k2*2N + i*N + t."""
    N = xT.shape[1]
    return np.ascontiguousarray(
        xT.reshape(4, 2, 128, N).transpose(2, 0, 1, 3).reshape(128, 8 * N)
    ).astype(E4)


def _prep_shards(x, Wq, Wk, Wv, Wproj, bproj, ln1_g, ln1_b, ln2_g, ln2_b, W1, b1, W2, b2):
    Wqf = np.asarray(Wq).transpose(1, 0, 2).reshape(C, C)
    Wkf = np.asarray(Wk).transpose(1, 0, 2).reshape(C, C)
    Wvf = np.asarray(Wv).transpose(1, 0, 2).reshape(C, C)
    g1 = np.asarray(ln1_g, np.float32)[:, None]
    g2 = np.asarray(ln2_g, np.float32)[:, None]
    # fold gamma into the projection weights (exact LN fusing)
    Wq_g, Wk_g, Wv_g = Wqf * g1, Wkf * g1, Wvf * g1
    W1_g = np.asarray(W1, np.float32) * g2
    wq8, sq = _pack_dr(Wq_g)
    wk8, sk = _pack_dr(Wk_g)
    wv = _pack_rows(Wv_g)
    wp = np.ascontiguousarray(
        np.asarray(Wproj).reshape(8, 128, 8, 128).transpose(1, 2, 0, 3).reshape(128, C * 8)
    ).astype(BF)
    w1 = np.ascontiguousarray(
        W1_g.reshape(8, 128, 8, 512).transpose(1, 2, 0, 3).reshape(128, 8 * DFF)
    ).astype(BF)
    w2 = _pack_rows(np.asarray(W2))
    sqk = np.array([[sq, sk]], np.float32)
    # colsum rows from fp8/bf16-rounded weights (must match on-chip matmuls)
    csq = wq8.astype(np.float32).reshape(128, 4, 2, C).sum(axis=(0, 1, 2))
    csk = wk8.astype(np.float32).reshape(128, 4, 2, C).sum(axis=(0, 1, 2))
    csv = wv.astype(np.float32).reshape(128, 8, C).sum(axis=(0, 1))
    # w1 pack col = db*4096 + k*512 + jj -> colsum over (p, k) for each (db, jj)
    cs1 = np.ascontiguousarray(
        w1.astype(np.float32).reshape(128, 8, 8, 512).sum(axis=(0, 2)).reshape(DFF))
    cs = csv.astype(BF)[None, :]
    # colsum COLUMNS for the write-op rank-1 correction: [cskc | csqc | cs1c]
    csc = np.concatenate([
        csk.reshape(8, 128).T, csq.reshape(8, 128).T, cs1.reshape(32, 128).T,
    ], axis=1).astype(BF)
    # beta rows: added into the psum as std*(beta@W)/wscale before the rstd mul
    b1v = np.asarray(b1, np.float32)
    bw = np.concatenate([
        (np.asarray(ln1_b, np.float32) @ Wqf) / sq,
        (np.asarray(ln1_b, np.float32) @ Wkf) / sk,
        np.asarray(ln1_b, np.float32) @ Wvf,
        np.asarray(ln2_b, np.float32) @ np.asarray(W1, np.float32) + b1v,
    ]).astype(BF)[None, :]
    bpT = np.ascontiguousarray(
        np.stack([bproj, b2]).reshape(2, 8, 128).transpose(2, 0, 1).reshape(128, 16)
    ).astype(np.float32)
    ident = np.eye(128, dtype=np.float32).astype(BF)

    in_maps = []
    for c in range(8):
        bidx, sub = c // 2, c % 2
        (lo0, lo1), (hi0, hi1) = own_ranges(sub)
        xb_ = np.asarray(x[bidx])
        x_own = np.concatenate([xb_[lo0:lo1], xb_[hi0:hi1]], axis=0)
        keys = np.arange(T)
        rows_b0 = np.arange(lo0, lo1)
        rows_b1 = np.arange(hi0, hi1)
        m = np.zeros((T, 512), np.float32)
        m[0:1024] = (keys[0:1024, None] <= rows_b0[None, :])
        m[1024:2048] = (keys[1024:2048, None] <= rows_b1[None, :])
        madd = -1000.0 * (1.0 - m)
        maskp = np.ascontiguousarray(
            madd.reshape(16, 128, 512).transpose(1, 0, 2).reshape(128, 16 * 512)
        ).astype(BF)
        xT = np.ascontiguousarray(xb_.T).astype(np.float32)
        xqT = np.ascontiguousarray(x_own.T).astype(np.float32)
        in_maps.append({
            "x8": _pack_pairs_fp8(xT),
            "x8q": _pack_pairs_fp8(xqT),
            "xb": _pack_rows(xT),
            "xTq": xqT,
            "maskp": maskp, "ident": ident,
            "wq8": wq8, "wk8": wk8, "wv": wv, "wp": wp,
            "w1": w1, "w2": w2, "cs": cs, "bw": bw, "sqk": sqk, "csc": csc,
            "bpT": bpT,
        })
    return in_maps


def kernel(repeat: int = 1, loop_n: int = 0, variant: str = "", **inputs) -> np.ndarray:
    inputs = {k: np.asarray(v) for k, v in inputs.items()}
    apply_ln_affine = not (
        np.all(inputs["ln1_b"] == 0) and np.all(inputs["ln2_b"] == 0)
        and np.all(inputs["b1"] == 0))
    add_bproj = bool(np.any(inputs["bproj"] != 0))
    add_b2 = bool(np.any(inputs["b2"] != 0))
    nc = _cached_program(apply_ln_affine, add_bproj, add_b2, repeat, loop_n, variant)
    in_maps = _prep_shards(**inputs)
    res = run_bass_kernel_spmd(nc, in_maps, list(range(8)))
    out = np.empty((B, T, C), np.float32)
    for c in range(8):
        bidx, sub = c // 2, c % 2
        (lo0, lo1), (hi0, hi1) = own_ranges(sub)
        oc = np.asarray(res.results[c]["out"]).T  # [R, C]
        out[bidx, lo0:lo1] = oc[0:512]
        out[bidx, hi0:hi1] = oc[512:1024]
    return out
